# revision 1
# baseline (speedup 1.0000x reference)
"""Block-sparse attention on 8 TRN2 NeuronCores.

Sharding: head-parallel (8 heads -> 8 cores). Each core computes one head's
q/k/v projections + block-sparse attention in a transposed layout (scores
kept as S^T tiles [j_part, i_free] so softmax sums come from an appended
ones-column in v-hat and no P transposes are needed), then an AllToAll
redistributes per-head outputs into row-shards so every core computes its
512-row slice of the final out-projection. Host only reshapes/concats.

Sparsity schedule is derived at runtime from the actual mask (block-level
any/all at 64x64 granularity) so the kernel is correct for any mask; the
elementwise mask is applied on-device (per-partition 0/1 vectors where the
mask is constant along i inside a cell, full 64x64 0/1 tiles otherwise).
"""

import os

import numpy as np
import ml_dtypes

import bass_rust
import concourse.bass as bass
import concourse.mybir as mybir
import concourse.tile as tile
from concourse.bass_utils import run_bass_kernel_spmd
from concourse.masks import make_identity


def split_multi_waits(nc):
    """Walrus in this toolchain embeds at most ONE sync-wait per instruction
    (every ISA struct has a single `events` slot). Tile's add_semaphores can
    attach several. Legalize: hoist all but the last wait of any instruction
    onto standalone EventSemaphore ops on the same engine, placed just before
    it — the engine executes its stream in order, so semantics are identical.
    """
    n = 0
    for bb in nc.m.functions[0].blocks:
        insts = bb.instructions
        out = []
        for inst in insts:
            si = inst.sync_info
            if si is not None and len(si.on_wait) > 1:
                waits = [bass_rust.SyncWait(sync_type=w.sync_type, id=w.id,
                                            ant_name=w.ant_name,
                                            wait_mode=w.wait_mode,
                                            wait_value=w.wait_value,
                                            wait_reg=w.wait_reg)
                         for w in si.on_wait]
                for w in waits[:-1]:
                    es = mybir.InstEventSemaphore(name=f"eswx_{n}")
                    n += 1
                    es.engine = inst.engine
                    es.sync_info = bass_rust.SyncInfo(on_wait=[w], on_update=[])
                    out.append(es)
                ups = [bass_rust.SyncUpdate(sync_type=u.sync_type, id=u.id,
                                            ant_name=u.ant_name,
                                            update_mode=u.update_mode,
                                            update_value=u.update_value)
                       for u in si.on_update]
                inst.sync_info = bass_rust.SyncInfo(on_wait=[waits[-1]],
                                                    on_update=ups)
            out.append(inst)
        insts[:] = out
    return n

F32 = mybir.dt.float32
F32R = mybir.dt.float32r
BF16 = mybir.dt.bfloat16
NPBF16 = ml_dtypes.bfloat16

T, DM, H, D, BS = 4096, 512, 8, 64, 64
NB = T // BS           # 64 block rows/cols
NCORES = 8
QS = 8                 # strips (64-row blocks) per accumulator window
QW = QS * BS           # 512 columns per output-accumulator window (1 PSUM bank)
NQ = NB // QS          # 8 windows
MAX_RUN = 8            # cap run length so N <= 512 (one PSUM bank)

LAST_RESULTS = None    # test harness reads exec_time_ns etc. from here

# tuning knobs
BUFS_S = 3     # QK psum tiles
BUFS_P = 12    # exp sbuf tiles
BUFS_ACC = 2   # output accumulators (per side)
BUFS_DV = 3    # division temporaries


# --------------------------------------------------------------------------
# host-side schedule derivation (pure restructuring of the mask input)
# --------------------------------------------------------------------------

def build_structure(mask):
    mask = np.asarray(mask).astype(bool).reshape(T, T)
    mb = mask.reshape(NB, BS, NB, BS).transpose(0, 2, 1, 3)  # [br, jb, i, j]
    anyb = mb.any(axis=(2, 3))
    fullb = mb.all(axis=(2, 3))

    vecs = {}    # bytes -> idx ; per-partition 0/1 column [j]
    tiles = {}   # bytes -> idx ; transposed 0/1 tile [j, i]
    cellinfo = {}
    for br in range(NB):
        for jb in range(NB):
            if not anyb[br, jb]:
                continue
            if fullb[br, jb]:
                cellinfo[(br, jb)] = ("full", 0)
                continue
            cell = mb[br, jb]  # [i, j]
            if (cell == cell[0:1]).all():
                key = cell[0].tobytes()
                idx = vecs.setdefault(key, len(vecs))
                cellinfo[(br, jb)] = ("vec", idx)
            else:
                key = np.ascontiguousarray(cell.T).tobytes()
                idx = tiles.setdefault(key, len(tiles))
                cellinfo[(br, jb)] = ("tile", idx)

    # pieces[(q)] = list of dicts: one QK matmul + exp slice + mask ops + PV
    pieces_by_q = [[] for _ in range(NQ)]
    for jb in range(NB):
        strips = [br for br in range(NB) if anyb[br, jb]]
        for q in range(NQ):
            sq = [s for s in strips if q * QS <= s < (q + 1) * QS]
            if not sq:
                continue
            runs, cur = [], [sq[0]]
            for s in sq[1:]:
                if s == cur[-1] + 1:
                    cur.append(s)
                else:
                    runs.append(cur)
                    cur = [s]
            runs.append(cur)
            singles = []
            final_runs = []
            for r in runs:
                while len(r) > MAX_RUN:
                    final_runs.append(r[:MAX_RUN])
                    r = r[MAX_RUN:]
                if len(r) >= 2:
                    final_runs.append(r)
                elif r:
                    singles.append(r[0])
            plist = []
            for r in final_runs:
                plist.append({"kind": "run", "strips": r})
            for i in range(0, len(singles) - 1, 2):
                plist.append({"kind": "pair", "strips": [singles[i], singles[i + 1]]})
            if len(singles) % 2:
                plist.append({"kind": "single", "strips": [singles[-1]]})
            for p in plist:
                p["jb"] = jb
                p["N"] = 64 * len(p["strips"])
                # mask ops: (off_in_piece, type, idx); coalesce all-same-vec
                ops = []
                kinds = [cellinfo[(s, jb)] for s in p["strips"]]
                if all(k[0] == "vec" for k in kinds) and len({k[1] for k in kinds}) == 1:
                    ops.append(("vec_all", kinds[0][1], 0))
                else:
                    for off, s in enumerate(p["strips"]):
                        t, idx = cellinfo[(s, jb)]
                        if t == "vec":
                            ops.append(("vec", idx, off))
                        elif t == "tile":
                            ops.append(("tile", idx, off))
                p["ops"] = ops
                pieces_by_q[q].append(p)

    # pair pieces (even jb -> partitions 0:64, odd jb -> 64:128) of equal N
    groups_by_q = []
    for q in range(NQ):
        lo = [p for p in pieces_by_q[q] if p["jb"] % 2 == 0]
        hi = [p for p in pieces_by_q[q] if p["jb"] % 2 == 1]
        lo.sort(key=lambda p: -p["N"])
        hi.sort(key=lambda p: -p["N"])
        groups = []
        while lo and hi:
            a = lo[0]
            b = next((x for x in hi if x["N"] == a["N"]), None)
            if b is None:
                groups.append((lo.pop(0), None))
            else:
                hi.remove(b)
                groups.append((lo.pop(0), b))
        groups.extend((p, None) for p in lo)
        groups.extend((None, p) for p in hi)
        groups_by_q.append(groups)

    vec_arr = np.zeros((BS, max(1, len(vecs))), dtype=np.float32)
    for key, idx in vecs.items():
        vec_arr[:, idx] = np.frombuffer(key, dtype=bool).astype(np.float32)
    tile_arr = np.zeros((BS, max(1, len(tiles)) * BS), dtype=NPBF16)
    for key, idx in tiles.items():
        tl = np.frombuffer(key, dtype=bool).reshape(BS, BS).astype(NPBF16)
        tile_arr[:, idx * BS:(idx + 1) * BS] = tl

    return {
        "groups_by_q": groups_by_q,
        "vec_arr": vec_arr,
        "tile_arr": tile_arr,
        "nv": vec_arr.shape[1],
        "nu": tile_arr.shape[1] // BS,
    }


# --------------------------------------------------------------------------
# device program
# --------------------------------------------------------------------------

def _ap3(base_ap, stride_elems):
    """[P, 64] slice -> [P, 2, 64] with the given free-axis stride."""
    a = base_ap
    return bass.AP(tensor=a.tensor, offset=a.offset,
                   ap=[list(a.ap[0]), [stride_elems, 2], [1, 64]])


def build_program(st, legalize=True, phase="ABC"):
    nc = bass.Bass("TRN2", target_bir_lowering=False, debug=False,
                   num_devices=NCORES)
    xT = nc.dram_tensor("xT", [DM, T], BF16, kind="ExternalInput").ap()
    w1 = nc.dram_tensor("w1", [DM, 128], BF16, kind="ExternalInput").ap()
    w2 = nc.dram_tensor("w2", [DM, 128], BF16, kind="ExternalInput").ap()
    woT = nc.dram_tensor("woT", [DM, DM], F32, kind="ExternalInput").ap()
    mvec = nc.dram_tensor("mvec", [BS, st["nv"]], F32, kind="ExternalInput").ap()
    mtile = nc.dram_tensor("mtile", [BS, st["nu"] * BS], BF16,
                           kind="ExternalInput").ap()
    out_rows = nc.dram_tensor("out_rows", [T // NCORES, DM], F32,
                              kind="ExternalOutput").ap()

    with tile.TileContext(nc, pool_alloc_mode="queue") as tc:
        if phase.startswith("LOOP"):
            n_iter = int(phase[4:])
            with tc.For_i(0, n_iter, 1, hint_engines=(
                    mybir.EngineType.PE, mybir.EngineType.Activation,
                    mybir.EngineType.DVE, mybir.EngineType.SP)):
                _emit(tc, st, xT, w1, w2, woT, mvec, mtile, out_rows,
                      phase="NL")
        else:
            _emit(tc, st, xT, w1, w2, woT, mvec, mtile, out_rows, phase=phase)
    if legalize:
        split_multi_waits(nc)
    return nc


def _emit(tc, st, xT, w1, w2, woT, mvec, mtile, out_rows, phase="ABC"):
    nc = tc.nc
    Exp = mybir.ActivationFunctionType.Exp
    NV, NU = st["nv"], st["nu"]

    persist = tc.alloc_tile_pool(name="persist", bufs=1)
    dram = tc.alloc_tile_pool(name="dram", bufs=1, space="DRAM")

    # ---------------- persistent SBUF ----------------
    xts = [persist.tile([128, T], BF16, tag=f"xt{k}", name=f"xt{k}") for k in range(4)]
    AB1 = persist.tile([128, T], BF16, tag="ab1", name="ab1")   # [kT ; qT] halves
    AB2 = persist.tile([128, T], BF16, tag="ab2", name="ab2")   # [vT ; kT] halves
    QLO = persist.tile([64, T], BF16, tag="qlo", name="qlo")
    vhat = persist.tile([128, NB // 2, D + 1], BF16, tag="vhat", name="vhat")
    outT = persist.tile([64, T], F32, tag="outT", name="outT")
    w1s = persist.tile([128, 4, 128], BF16, tag="w1s", name="w1s")
    w2s = persist.tile([128, 4, 128], BF16, tag="w2s", name="w2s")
    wos = persist.tile([128, 4, DM], F32R, tag="wos", name="wos")
    mv = persist.tile([128, NV], F32, tag="mv", name="mv")
    mt = persist.tile([128, NU, BS], BF16, tag="mt", name="mt")
    ident = persist.tile([64, 64], BF16, tag="ident", name="ident")
    ones_t = persist.tile([65, 64], BF16, tag="ones", name="ones")
    zc = persist.tile([1, 64 + 1 + QW], BF16, tag="zc", name="zc")

    a2a_in = dram.tile([DM, T // NCORES], F32)
    a2a_out = dram.tile([DM, T // NCORES], F32)

    # ---------------- loads + constants ----------------
    for k in range(4):
        nc.sync.dma_start(out=xts[k][:, :], in_=xT[128 * k:128 * (k + 1), :])
    nc.sync.dma_start(out=w1s[:, :, :],
                      in_=w1.rearrange("(c p) m -> p c m", p=128))
    nc.sync.dma_start(out=w2s[:, :, :],
                      in_=w2.rearrange("(c p) m -> p c m", p=128))
    nc.sync.dma_start(out=wos[:, :, :],
                      in_=woT.rearrange("(c p) m -> p c m", p=128).bitcast(F32R))
    nc.sync.dma_start(out=mv[0:64, :], in_=mvec[:, :])
    nc.sync.dma_start(out=mv[64:128, :], in_=mvec[:, :])
    nc.sync.dma_start(out=mt[0:64, :, :],
                      in_=mtile.rearrange("j (u i) -> j u i", i=BS))
    nc.sync.dma_start(out=mt[64:128, :, :],
                      in_=mtile.rearrange("j (u i) -> j u i", i=BS))
    make_identity(nc, ident[:, :])
    nc.vector.memset(ones_t[:, :], 1.0)
    nc.vector.memset(zc[:, :], 0.0)
    nc.vector.memset(vhat[:, :, :], 1.0)

    # ---------------- phase A: projections (K=512 in 4 chunks) ----------------
    with tc.tile_pool(name="psA", bufs=2, space="PSUM") as psA, \
         tc.tile_pool(name="psT", bufs=2, space="PSUM") as psT:
        for pi, (ws, AB) in enumerate(((w1s, AB1), (w2s, AB2))):
            for n in range(8):
                ps = psA.tile([128, 512], F32, tag="proj", name="proj")
                for k in range(4):
                    nc.tensor.matmul(ps[:, :], ws[:, k, :],
                                     xts[k][:, 512 * n:512 * (n + 1)],
                                     start=(k == 0), stop=(k == 3))
                dst = AB[:, 512 * n:512 * (n + 1)]
                if (pi * 8 + n) % 2 == 0:
                    nc.scalar.copy(dst, ps[:, :])
                else:
                    nc.vector.tensor_copy(dst, ps[:, :])
        # q into partitions 0:64 (copy of AB1's upper half)
        for n in range(8):
            nc.sync.dma_start(out=QLO[:, 512 * n:512 * (n + 1)],
                              in_=AB1[64:128, 512 * n:512 * (n + 1)])
        # v-hat: transpose vT (AB2 lower half) into natural [j, d] layout
        for t in range(NB // 2):
            pt = psT.tile([128, 64], BF16, tag="vt", name="vt")
            nc.tensor.transpose(pt[:, :], AB2[0:64, 128 * t:128 * (t + 1)],
                                ident[:, :])
            nc.vector.tensor_copy(vhat[:, t, 0:D], pt[:, :])

    if phase == "A":
        fo0 = persist.tile([128, DM], F32, tag="fo0", name="fo0")
        nc.vector.tensor_copy(fo0[:, :], AB1[:, 0:DM])
        for s in range(4):
            nc.sync.dma_start(out=out_rows[128 * s:128 * (s + 1), :],
                              in_=fo0[:, :])
        dram.release()
        persist.release()
        return

    # ---------------- phase B: sparse attention ----------------
    with tc.tile_pool(name="psS", bufs=BUFS_S, space="PSUM") as psS, \
         tc.tile_pool(name="psO", bufs=BUFS_ACC, space="PSUM") as psO, \
         tc.tile_pool(name="psB", bufs=1, space="PSUM") as psB, \
         tc.tile_pool(name="pp", bufs=BUFS_P) as pp, \
         tc.tile_pool(name="dv", bufs=BUFS_DV) as dv:
        oct_range = range(1) if phase == "B1" else range(NQ)
        for q in oct_range:
            # separate accumulators per partition side: concurrent row-group
            # matmuls into overlapping PSUM regions are fatal on HW
            acc_lo = psO.tile([65, QW], F32, tag="accl", name="accl")
            acc_hi = psO.tile([65, QW], F32, tag="acch", name="acch")
            nc.tensor.matmul(acc_lo[:, :], zc[0:1, 0:65], zc[0:1, 65:65 + QW],
                             start=True, stop=False, skip_group_check=True)
            nc.tensor.matmul(acc_hi[:, :], zc[0:1, 0:65], zc[0:1, 65:65 + QW],
                             start=True, stop=False, skip_group_check=True)
            groups = st["groups_by_q"][q]
            n_groups = len(groups)
            for gi, (plo, phi) in enumerate(groups):
                N = (plo or phi)["N"]
                s_ps = psS.tile([128, N], F32, tag="s", name="s")
                p_sb = pp.tile([128, N], BF16, tag="p", name="p")
                last_in_q = gi == n_groups - 1
                for p in (plo, phi):
                    if p is None:
                        continue
                    jb = p["jb"]
                    base = (jb % 2) * 64
                    ksrc = AB1 if base == 0 else AB2
                    qsrc = QLO[0:64, :] if base == 0 else AB1[64:128, :]
                    lhs_k = ksrc[base:base + 64, BS * jb:BS * (jb + 1)]
                    sl = p["strips"]
                    if p["kind"] in ("run", "single"):
                        rhs = qsrc[:, BS * sl[0]:BS * (sl[0] + len(sl))]
                    else:
                        rhs = _ap3(qsrc[:, BS * sl[0]:BS * sl[0] + BS],
                                   BS * (sl[1] - sl[0]))
                    nc.tensor.matmul(s_ps[base:base + 64, :], lhs_k, rhs,
                                     start=True, stop=True)
                # exp (scores scale 1/sqrt(D) folded in)
                if plo is not None and phi is not None:
                    nc.scalar.activation(p_sb[:, :], s_ps[:, :], Exp,
                                         scale=0.125)
                else:
                    base = 0 if plo is not None else 64
                    nc.scalar.activation(p_sb[base:base + 64, :],
                                         s_ps[base:base + 64, :], Exp,
                                         scale=0.125)
                # element masks
                for p in (() if phase == "Bnm" else (plo, phi)):
                    if p is None:
                        continue
                    base = (p["jb"] % 2) * 64
                    for kind, idx, off in p["ops"]:
                        if kind == "vec_all":
                            nc.vector.tensor_scalar_mul(
                                p_sb[base:base + 64, :],
                                p_sb[base:base + 64, :],
                                mv[base:base + 64, idx:idx + 1])
                        elif kind == "vec":
                            slc = p_sb[base:base + 64, BS * off:BS * (off + 1)]
                            nc.vector.tensor_scalar_mul(
                                slc, slc, mv[base:base + 64, idx:idx + 1])
                        else:
                            slc = p_sb[base:base + 64, BS * off:BS * (off + 1)]
                            nc.vector.tensor_mul(
                                slc, slc, mt[base:base + 64, idx, :])
                # PV accumulate into the side's accumulator
                for p in (() if phase == "Bnp" else (plo, phi)):
                    if p is None:
                        continue
                    jb = p["jb"]
                    base = (jb % 2) * 64
                    acc = acc_lo if base == 0 else acc_hi
                    sl = p["strips"]
                    lhs_v = vhat[base:base + 64, jb // 2, :]
                    stop = last_in_q and p is (phi if phi is not None else plo)
                    if p["kind"] in ("run", "single"):
                        o = BS * (sl[0] - q * QS)
                        nc.tensor.matmul(acc[:, o:o + p["N"]], lhs_v,
                                         p_sb[base:base + 64, :],
                                         start=False, stop=stop,
                                         skip_group_check=True)
                    else:
                        for si, s in enumerate(sl):
                            o = BS * (s - q * QS)
                            nc.tensor.matmul(
                                acc[:, o:o + BS], lhs_v,
                                p_sb[base:base + 64, BS * si:BS * (si + 1)],
                                start=False, stop=stop and si == 1,
                                skip_group_check=True)
            # combine side accumulators, divide by row-sums
            hi_sb = dv.tile([65, QW], F32, tag="his", name="his")
            nc.scalar.copy(hi_sb[:, :], acc_hi[:, :])
            sum_sb = dv.tile([65, QW], F32, tag="sums", name="sums")
            nc.vector.tensor_add(sum_sb[:, :], acc_lo[:, :], hi_sb[:, :])
            if phase in ("Bnd", "Bnp"):
                nc.scalar.copy(outT[:, QW * q:QW * (q + 1)], sum_sb[0:64, :])
                continue
            nc.vector.tensor_scalar_add(sum_sb[64:65, :], sum_sb[64:65, :],
                                        1e-30)
            rc = dv.tile([65, QW], BF16, tag="rc", name="rc")
            with nc.allow_low_precision(reason="bf16 recip for PE broadcast"):
                nc.vector.reciprocal(rc[64:65, :], sum_sb[64:65, :])
            bc_ps = psB.tile([64, QW], F32, tag="bc", name="bc")
            nc.tensor.matmul(bc_ps[:, :], ones_t[64:65, 0:64],
                             rc[64:65, :], start=True, stop=True)
            bc = dv.tile([64, QW], F32, tag="bcs", name="bcs")
            nc.scalar.copy(bc[:, :], bc_ps[:, :])
            nc.vector.tensor_mul(outT[:, QW * q:QW * (q + 1)],
                                 sum_sb[0:64, :], bc[:, :])

    if phase.startswith("B") or phase == "AB":
        fo0 = persist.tile([128, DM], F32, tag="fo0", name="fo0")
        nc.vector.tensor_copy(fo0[0:64, :], outT[:, 0:DM])
        for s in range(4):
            nc.sync.dma_start(out=out_rows[128 * s:128 * (s + 1), :],
                              in_=fo0[:, :])
        dram.release()
        persist.release()
        return

    # ---------------- phase C: AllToAll + out-projection row shard ----------
    RQ = T // NCORES  # 512
    for c in range(NCORES):
        nc.sync.dma_start(out=a2a_in[64 * c:64 * (c + 1), :],
                          in_=outT[:, RQ * c:RQ * (c + 1)])
    if phase == "NL":  # timeline-sim variant: no collective
        for r in range(0, DM, 128):
            nc.sync.dma_start(out=a2a_out[r:r + 128, :],
                              in_=a2a_in[r:r + 128, :])
    else:
        nc.gpsimd.collective_compute(
            "AllToAll", mybir.AluOpType.bypass,
            replica_groups=[list(range(NCORES))],
            ins=[a2a_in[:, :].opt()], outs=[a2a_out[:, :].opt()])

    with tc.tile_pool(name="psF", bufs=2, space="PSUM") as psF, \
         tc.tile_pool(name="fin", bufs=4) as fin:
        cts = []
        for k in range(4):
            ct = fin.tile([128, RQ], F32R, tag=f"ct{k}", name=f"ct{k}")
            nc.sync.dma_start(out=ct[:, :],
                              in_=a2a_out[128 * k:128 * (k + 1), :].bitcast(F32R))
            cts.append(ct)
        for s in range(4):
            ps = psF.tile([128, DM], F32, tag="fo", name="fo")
            for k in range(4):
                nc.tensor.matmul(ps[:, :], cts[k][:, 128 * s:128 * (s + 1)],
                                 wos[:, k, :], start=(k == 0), stop=(k == 3))
            fo = fin.tile([128, DM], F32, tag="fos", name="fos")
            nc.scalar.copy(fo[:, :], ps[:, :])
            nc.sync.dma_start(out=out_rows[128 * s:128 * (s + 1), :],
                              in_=fo[:, :])

    dram.release()
    persist.release()


# --------------------------------------------------------------------------
# entry point
# --------------------------------------------------------------------------

_CACHE = {}


def _get_program(mask, legalize=True, phase="ABC"):
    key = (np.asarray(mask).tobytes(), legalize, phase)
    if _CACHE.get("key") != key:
        st = build_structure(mask)
        _CACHE["key"] = key
        _CACHE["st"] = st
        _CACHE["nc"] = build_program(st, legalize=legalize, phase=phase)
    return _CACHE["nc"], _CACHE["st"]


def _make_in_maps(inputs, st):
    x, Wq, Wk, Wv, Wo = (np.asarray(inputs[k])
                         for k in ("x", "Wq", "Wk", "Wv", "Wo"))
    xT = np.ascontiguousarray(x.reshape(T, DM).T).astype(NPBF16)
    woT = np.ascontiguousarray(Wo.T).astype(np.float32)
    in_maps = []
    for h in range(NCORES):
        sl = slice(D * h, D * (h + 1))
        w1 = np.ascontiguousarray(
            np.concatenate([Wk[sl].T, Wq[sl].T], axis=1)).astype(NPBF16)
        w2 = np.ascontiguousarray(
            np.concatenate([Wv[sl].T, Wk[sl].T], axis=1)).astype(NPBF16)
        in_maps.append({
            "xT": xT, "w1": w1, "w2": w2, "woT": woT,
            "mvec": st["vec_arr"], "mtile": st["tile_arr"],
        })
    return in_maps


def kernel(x, mask, Wq, Wk, Wv, Wo, trace=False):
    global LAST_RESULTS
    x = np.asarray(x)
    Wq, Wk, Wv, Wo = (np.asarray(a) for a in (Wq, Wk, Wv, Wo))
    nc, st = _get_program(mask)

    in_maps = _make_in_maps(
        {"x": x, "Wq": Wq, "Wk": Wk, "Wv": Wv, "Wo": Wo}, st)

    try:
        res = run_bass_kernel_spmd(nc, in_maps, core_ids=list(range(NCORES)),
                                   trace=trace)
    except ModuleNotFoundError:
        # this container lacks the axon NTFF profile hook; retry untraced
        os.environ["BASS_NEVER_TRACE"] = "1"
        res = run_bass_kernel_spmd(nc, in_maps, core_ids=list(range(NCORES)),
                                   trace=False)
    LAST_RESULTS = res
    out = np.concatenate([res.results[c]["out_rows"] for c in range(NCORES)],
                         axis=0)
    return out.reshape(1, T, DM).astype(np.float32)

